# revision 11
# baseline (speedup 1.0000x reference)
"""ConvNeXt-GNN (kNN graph + 2 GCN blocks + classifier) Trainium2 Bass kernel.

Data-parallel over batch: 64 samples -> 8 cores x 8 samples.

Design (per sample, all on-chip after the token load):
  * Residual stream x kept FEATURE-major ([1024 feat (8x128 part-chunks), 256 nodes])
    so every matmul avoids activation transposes:
      - kNN scores S = X X^T straight from the feature-major tiles (float32r).
      - h @ W uses h-feature-major as lhsT -> node-major out;
        A_hat @ Y flips orientation back via A_hat's symmetry (Y as lhsT, A as rhs).
  * LN stats via ones-matmuls on PE (cross-partition reduce); rstd computed with
    a pure-DVE Newton rsqrt on [128,2] node-major columns (obtained by a PE
    transpose of the broadcast stats), so ACT only ever uses the
    gelu_and_others table set (gelu/copy/square) -> one table load total.
  * LN affine (g,b) folded into W1 / handled by ACT scale+bias; mean-shift of
    the pre-matmul LN folded into a rank-1 K=1 matmul correction (m x colsum(W1)).
  * top-8 neighbours via the DVE max8 unit + per-row threshold compare;
    A_hat assembled with PE transposes (symmetrize) + Newton rsqrt degree scaling.

kernel(**inputs) -> np.ndarray [64, 1000] float32.
"""

import sys

for _p in ("/opt/trn_rl_repo",):
    if _p not in sys.path:
        sys.path.append(_p)

import numpy as np
import ml_dtypes

import concourse.bass as bass
import concourse.tile as tile
from concourse import mybir, bacc
from concourse.bass_utils import run_bass_kernel_spmd

F32 = mybir.dt.float32
F32R = mybir.dt.float32r
BF16 = mybir.dt.bfloat16
AF = mybir.ActivationFunctionType
OP = mybir.AluOpType

B, N, D, HID, NB, NC_ = 64, 256, 1024, 512, 2, 1000
EPS = 1e-5
NCORES = 8
SPC = B // NCORES          # samples per core
DC = D // 128              # feature chunks (8)
HC = HID // 128            # hidden chunks (4)
NT = N // 128              # node tiles (2)
BIGNEG = -1.0e30

_CACHE = {}


def _round_f32r(x: np.ndarray) -> np.ndarray:
    """Round fp32 to the fp32r grid (e8m11 in the top 20 bits), RNE."""
    b = np.ascontiguousarray(x, dtype=np.float32).view(np.uint32).astype(np.uint64)
    tail = b & 0xFFF
    hi = b >> 12
    add = (tail > 0x800) | ((tail == 0x800) & ((hi & 1) == 1))
    hi = hi + add.astype(np.uint64)
    return ((hi << 12) & 0xFFFFFFFF).astype(np.uint32).view(np.float32)


def _newton_rsqrt(nc, pool, v_sb, out_sb, seed_a, seed_b, iters):
    """out = 1/sqrt(v) elementwise on a small [128, k] fp32 SBUF tile.

    Seed y0 = seed_a/v + seed_b (one reciprocal + one tensor_scalar), then
    Newton y' = y(1.5 - 0.5 v y^2). Pure DVE -- no ACT table needed.
    """
    shp = list(v_sb.shape)
    rec = pool.tile(shp, F32, name="nr_rec", tag="nr_rec")
    nc.vector.reciprocal(out=rec, in_=v_sb)
    y = pool.tile(shp, F32, name="nr_y", tag="nr_y")
    nc.vector.tensor_scalar(out=y, in0=rec, scalar1=float(seed_a), scalar2=float(seed_b),
                            op0=OP.mult, op1=OP.add)
    for _ in range(iters):
        t = pool.tile(shp, F32, name="nr_t", tag="nr_t")
        nc.vector.tensor_tensor(out=t, in0=y, in1=y, op=OP.mult)
        nc.vector.tensor_tensor(out=t, in0=t, in1=v_sb, op=OP.mult)
        nc.vector.tensor_scalar(out=t, in0=t, scalar1=-0.5, scalar2=1.5,
                                op0=OP.mult, op1=OP.add)
        nc.vector.tensor_tensor(out=y, in0=y, in1=t, op=OP.mult)
    nc.vector.tensor_copy(out_sb, y)


def build_kernel():
    nc = bacc.Bacc("TRN2")

    xT_in = nc.declare_dram_parameter("xT", [SPC, D, N], F32R, isOutput=False)
    w1f_in = nc.declare_dram_parameter("w1f", [128, NB, DC, HID], BF16, isOutput=False)
    w2_in = nc.declare_dram_parameter("w2", [128, NB, HC, D], BF16, isOutput=False)
    wc_in = nc.declare_dram_parameter("wc", [128, DC, NC_], BF16, isOutput=False)
    w1bn_in = nc.declare_dram_parameter("w1bn", [NB, HID], BF16, isOutput=False)
    eyebig_in = nc.declare_dram_parameter("eyebig", [128, NT, N], F32, isOutput=False)
    eyea_in = nc.declare_dram_parameter("eyea", [128, NT, N], mybir.dt.uint8, isOutput=False)
    i128b_in = nc.declare_dram_parameter("i128b", [128, 128], BF16, isOutput=False)
    i128f_in = nc.declare_dram_parameter("i128f", [128, 128], F32, isOutput=False)

    out_d = nc.declare_dram_parameter("out", [SPC, NC_], F32, isOutput=True)

    with tile.TileContext(nc) as tc:
        with (
            tc.tile_pool(name="wp", bufs=1) as wp,
            tc.tile_pool(name="xp", bufs=2) as xp,
            tc.tile_pool(name="sp", bufs=2) as sp,
            tc.tile_pool(name="tp", bufs=4) as tp,     # tiny tiles
            tc.tile_pool(name="pa", bufs=2, space="PSUM") as pa,
            tc.tile_pool(name="pstat", bufs=2, space="PSUM") as pstat,
            tc.tile_pool(name="pb", bufs=2, space="PSUM") as pb,
            tc.tile_pool(name="ph", bufs=2, space="PSUM") as ph,
        ):
            # ---- resident weights/constants ----
            w1f = wp.tile([128, NB, DC, HID], BF16)
            nc.sync.dma_start(out=w1f, in_=w1f_in[:, :, :, :])
            w2 = wp.tile([128, NB, HC, D], BF16)
            nc.sync.dma_start(out=w2, in_=w2_in[:, :, :, :])
            wc = wp.tile([128, DC, NC_], BF16)
            nc.sync.dma_start(out=wc, in_=wc_in[:, :, :])
            w1bn = wp.tile([1, NB, HID], BF16)
            nc.sync.dma_start(out=w1bn, in_=w1bn_in[None, :, :])
            eyebig = wp.tile([128, NT, N], F32)
            nc.sync.dma_start(out=eyebig, in_=eyebig_in[:, :, :])
            eyea = wp.tile([128, NT, N], mybir.dt.uint8)
            nc.sync.dma_start(out=eyea, in_=eyea_in[:, :, :])
            i128b = wp.tile([128, 128], BF16)
            nc.sync.dma_start(out=i128b, in_=i128b_in[:, :])
            i128f = wp.tile([128, 128], F32)
            nc.sync.dma_start(out=i128f, in_=i128f_in[:, :])

            onesK = wp.tile([128, 1], BF16)   # 1/1024 (LN stats lhsT)
            nc.vector.memset(onesK, 1.0 / 1024.0)
            onesColB = wp.tile([1, 128], BF16)
            nc.vector.memset(onesColB, 1.0)
            one1f = wp.tile([1, 1], F32)
            nc.vector.memset(one1f, 1.0)
            pooledT = wp.tile([128, DC, SPC], F32)

            def ln_stats(xstat, newton_iters=2):
                """Generator. xstat: [128, DC, 2, N] bf16, [:,c,0,:]=x, [:,c,1,:]=x^2
                (the x^2 half is computed here). Returns (mrs_sb [128,2N] bf16
                ([mean|rstd] row-bcast), m_row [1,N] bf16, stat_ps [1,2N] psum)."""
                stat_ps = pstat.tile([1, 2 * N], F32, name="stat_ps", tag="pstat")
                nc.vector.tensor_tensor(out=xstat[:, :, 1, :], in0=xstat[:, :, 0, :],
                                        in1=xstat[:, :, 0, :], op=OP.mult)
                for c in range(DC):
                    nc.tensor.matmul(stat_ps[0:1, :], lhsT=onesK, rhs=xstat[:, c, :, :],
                                     start=(c == 0), stop=(c == DC - 1))
                yield
                m_row = tp.tile([1, N], BF16, name="m_row", tag="m_row")
                nc.scalar.copy(out=m_row, in_=stat_ps[0:1, 0:N])
                msq = tp.tile([1, N], F32, name="msq", tag="msq")
                nc.vector.tensor_tensor(out=msq, in0=m_row, in1=m_row, op=OP.mult)
                veps_row = tp.tile([1, N], F32, name="veps_row", tag="veps_row")
                nc.vector.scalar_tensor_tensor(
                    out=veps_row, in0=stat_ps[0:1, N:2 * N], scalar=EPS, in1=msq,
                    op0=OP.add, op1=OP.subtract)
                vc_ps = ph.tile([128, NT], F32, name="vc_ps", tag="ph")
                for mt in range(NT):
                    nc.tensor.matmul(vc_ps[:, mt:mt + 1],
                                     lhsT=veps_row[0:1, mt * 128:(mt + 1) * 128],
                                     rhs=one1f, start=True, stop=True)
                veps_col = tp.tile([128, NT], F32, name="veps_col", tag="veps_col")
                nc.scalar.copy(out=veps_col, in_=vc_ps)
                rstd_col = tp.tile([128, NT], F32, name="rstd_col", tag="rstd_col")
                _newton_rsqrt(nc, tp, veps_col, rstd_col, 0.6, 0.3, newton_iters)
                yield
                mrs_ps = pb.tile([128, 2 * N], F32, name="mrs_ps", tag="pb")
                nc.tensor.matmul(mrs_ps[:, 0:N], lhsT=onesColB, rhs=m_row,
                                 start=True, stop=True)
                for mt in range(NT):
                    rsmat = sp.tile([128, 128], F32, name="rsmat", tag="rsmat")
                    nc.vector.tensor_copy(
                        rsmat, rstd_col[:, mt:mt + 1].broadcast_to([128, 128]))
                    nc.tensor.transpose(
                        mrs_ps[:, N + mt * 128:N + (mt + 1) * 128], rsmat, i128f)
                mrs_sb = sp.tile([128, 2 * N], BF16, name="mrs_sb", tag="mrs_sb")
                nc.scalar.copy(out=mrs_sb, in_=mrs_ps)
                return mrs_sb, m_row, stat_ps

            def sample_body(s):
                # ---- load (feature-major); xr is the pure-f32r kNN copy ----
                xr = xp.tile([128, DC, N], F32R, name="xr", tag="xr")
                nc.sync.dma_start(
                    out=xr, in_=xT_in[s].rearrange("(c p) n -> p c n", p=128))
                x = xp.tile([128, DC, N], F32R, name="x", tag="x", bufs=4)
                nc.sync.dma_start(
                    out=x, in_=xT_in[s].rearrange("(c p) n -> p c n", p=128))
                xf = x.bitcast(F32)
                xstat = sp.tile([128, DC, 2, N], BF16, name="xstat", tag="xstat", bufs=3)
                nc.vector.tensor_copy(xstat[:, :, 0, :], xf)
                yield
                mrs_sb, m_row, stat_ps = yield from ln_stats(xstat)
                sqneg = tp.tile([1, N], F32, name="sqneg", tag="sqneg")
                nc.scalar.activation(out=sqneg, in_=stat_ps[0:1, N:2 * N], func=AF.Copy,
                                     scale=-512.0)
                onesR = tp.tile([1, N], F32, name="onesR", tag="onesR")
                nc.vector.memset(onesR, 1.0)
                yield
                # ---- kNN scores & adjacency ----
                score = sp.tile([128, NT, N], F32, name="score", tag="score")
                top8 = tp.tile([128, NT, 8], F32, name="top8", tag="top8")
                a_bf = sp.tile([128, NT, N], BF16, name="a_bf", tag="a_bf", bufs=4)
                for mt in range(NT):
                    s_ps = pa.tile([128, N], F32, name="s_ps", tag="pa")
                    for c in range(DC):
                        nc.tensor.matmul(s_ps, lhsT=xr[:, c, mt * 128:(mt + 1) * 128],
                                         rhs=xr[:, c, :], start=(c == 0), stop=False)
                    nc.tensor.matmul(s_ps, lhsT=onesR[:, mt * 128:(mt + 1) * 128],
                                     rhs=sqneg, start=False, stop=False)
                    nc.tensor.matmul(s_ps, lhsT=sqneg[:, mt * 128:(mt + 1) * 128],
                                     rhs=onesR, start=False, stop=True)
                    nc.vector.tensor_tensor(out=score[:, mt, :], in0=s_ps,
                                            in1=eyebig[:, mt, :], op=OP.add)
                    nc.vector.max(out=top8[:, mt, :], in_=score[:, mt, :])
                    nc.vector.tensor_scalar(out=a_bf[:, mt, :], in0=score[:, mt, :],
                                            scalar1=top8[:, mt, 7:8], scalar2=None,
                                            op0=OP.is_ge)
                yield
                at_ps = []
                for mt in range(NT):
                    t_ps = pa.tile([128, N], BF16, name="at_ps", tag="pa")
                    for jt in range(NT):
                        nc.tensor.transpose(
                            t_ps[:, jt * 128:(jt + 1) * 128],
                            a_bf[:, jt, mt * 128:(mt + 1) * 128], i128b)
                    at_ps.append(t_ps)
                deg = tp.tile([128, NT], F32, name="deg", tag="deg")
                for mt in range(NT):
                    nc.vector.tensor_tensor(out=a_bf[:, mt, :], in0=a_bf[:, mt, :],
                                            in1=at_ps[mt], op=OP.max)
                    nc.vector.tensor_reduce(out=deg[:, mt:mt + 1], in_=a_bf[:, mt, :],
                                            axis=mybir.AxisListType.X, op=OP.add)
                dp1 = tp.tile([128, NT], F32, name="dp1", tag="dp1")
                nc.vector.tensor_scalar(out=dp1, in0=deg, scalar1=1.0, scalar2=None,
                                        op0=OP.add)
                dinv = tp.tile([128, NT], F32, name="dinv", tag="dinv")
                _newton_rsqrt(nc, tp, dp1, dinv, 2.5, 0.05, 3)
                for mt in range(NT):
                    nc.vector.tensor_scalar(out=a_bf[:, mt, :], in0=a_bf[:, mt, :],
                                            scalar1=dinv[:, mt:mt + 1], scalar2=None,
                                            op0=OP.mult)
                yield
                for mt in range(NT):
                    t_ps = pa.tile([128, N], BF16, name="a2_ps", tag="pa")
                    for jt in range(NT):
                        nc.tensor.transpose(
                            t_ps[:, jt * 128:(jt + 1) * 128],
                            a_bf[:, jt, mt * 128:(mt + 1) * 128], i128b)
                    nc.vector.tensor_scalar(out=a_bf[:, mt, :], in0=t_ps,
                                            scalar1=dinv[:, mt:mt + 1], scalar2=None,
                                            op0=OP.mult)
                dinv2 = tp.tile([128, NT], F32, name="dinv2", tag="dinv2")
                nc.vector.tensor_tensor(out=dinv2, in0=dinv, in1=dinv, op=OP.mult)
                for mt in range(NT):
                    nc.vector.copy_predicated(
                        out=a_bf[:, mt, :], mask=eyea[:, mt, :],
                        data=dinv2[:, mt:mt + 1].broadcast_to([128, N]))
                yield
                # ---- GCN blocks ----
                for i in range(NB):
                    if i > 0:
                        xstat = sp.tile([128, DC, 2, N], BF16, name="xstat", tag="xstat", bufs=3)
                        nc.vector.tensor_copy(xstat[:, :, 0, :], xf)
                        mrs_sb, m_row, _ = yield from ln_stats(xstat)
                    t_bf = sp.tile([128, DC, N], BF16, name="t_bf", tag="t_bf", bufs=3)
                    nc.vector.tensor_tensor(
                        out=t_bf, in0=xstat[:, :, 0, :],
                        in1=mrs_sb[:, N:2 * N].unsqueeze(1).broadcast_to([128, DC, N]),
                        op=OP.mult)
                    mrs = tp.tile([1, N], BF16, name="mrs", tag="mrs")
                    nc.vector.tensor_tensor(out=mrs, in0=m_row,
                                            in1=mrs_sb[0:1, N:2 * N], op=OP.mult)
                    yield
                    y1sb = sp.tile([128, NT, HID], BF16, name="y1sb", tag="y1sb")
                    for mt in range(NT):
                        y_ps = pb.tile([128, HID], F32, name="y_ps", tag="pb")
                        for c in range(DC):
                            nc.tensor.matmul(
                                y_ps, lhsT=t_bf[:, c, mt * 128:(mt + 1) * 128],
                                rhs=w1f[:, i, c, :], start=(c == 0), stop=False)
                        nc.tensor.matmul(y_ps, lhsT=mrs[:, mt * 128:(mt + 1) * 128],
                                         rhs=w1bn[:, i, :], start=False, stop=True)
                        if mt == 0:
                            nc.scalar.copy(out=y1sb[:, mt, :], in_=y_ps)
                        else:
                            nc.vector.tensor_copy(y1sb[:, mt, :], y_ps)
                    yield
                    z1sb = sp.tile([128, HC, N], BF16, name="z1sb", tag="z1sb")
                    for ht in range(HC):
                        z_ps = pa.tile([128, N], F32, name="z_ps", tag="pa")
                        for jt in range(NT):
                            nc.tensor.matmul(
                                z_ps, lhsT=y1sb[:, jt, ht * 128:(ht + 1) * 128],
                                rhs=a_bf[:, jt, :], start=(jt == 0), stop=(jt == NT - 1))
                        nc.scalar.activation(out=z1sb[:, ht, :], in_=z_ps, func=AF.Gelu)
                    yield
                    y2sb = sp.tile([128, NT, D], BF16, name="y2sb", tag="y2sb")
                    for mt in range(NT):
                        for nh in range(2):
                            y_ps = pb.tile([128, 512], F32, name="y2_ps", tag="pb")
                            for ht in range(HC):
                                nc.tensor.matmul(
                                    y_ps, lhsT=z1sb[:, ht, mt * 128:(mt + 1) * 128],
                                    rhs=w2[:, i, ht, nh * 512:(nh + 1) * 512],
                                    start=(ht == 0), stop=(ht == HC - 1))
                            if nh == 0:
                                nc.scalar.copy(
                                    out=y2sb[:, mt, nh * 512:(nh + 1) * 512], in_=y_ps)
                            else:
                                nc.vector.tensor_copy(
                                    y2sb[:, mt, nh * 512:(nh + 1) * 512], y_ps)
                    yield
                    h_sb = sp.tile([128, DC, N], BF16, name="h_sb", tag="h_sb")
                    for c in range(DC):
                        h_ps = ph.tile([128, N], F32, name="h_ps", tag="ph")
                        for jt in range(NT):
                            nc.tensor.matmul(
                                h_ps, lhsT=y2sb[:, jt, c * 128:(c + 1) * 128],
                                rhs=a_bf[:, jt, :], start=(jt == 0), stop=(jt == NT - 1))
                        nc.scalar.copy(out=h_sb[:, c, :], in_=h_ps)
                    nc.vector.tensor_tensor(out=xf, in0=xf, in1=h_sb, op=OP.add)
                    yield
                    xstat = sp.tile([128, DC, 2, N], BF16, name="xstat", tag="xstat", bufs=3)
                    nc.vector.tensor_copy(xstat[:, :, 0, :], xf)
                    mrs_sb, m_row, _ = yield from ln_stats(xstat)
                    t1 = sp.tile([128, DC, N], BF16, name="t_bf", tag="t_bf", bufs=3)
                    nc.vector.tensor_tensor(
                        out=t1, in0=xstat[:, :, 0, :],
                        in1=mrs_sb[:, 0:N].unsqueeze(1).broadcast_to([128, DC, N]),
                        op=OP.subtract)
                    nc.vector.tensor_tensor(
                        out=t1, in0=t1,
                        in1=mrs_sb[:, N:2 * N].unsqueeze(1).broadcast_to([128, DC, N]),
                        op=OP.mult)
                    gout = sp.tile([128, DC, N], BF16, name="gout", tag="gout")
                    nc.scalar.activation(out=gout, in_=t1, func=AF.Gelu)
                    nc.vector.tensor_tensor(out=xf, in0=xf, in1=gout, op=OP.add)
                    yield
                # ---- readout ----
                xstat = sp.tile([128, DC, 2, N], BF16, name="xstat", tag="xstat", bufs=3)
                nc.vector.tensor_copy(xstat[:, :, 0, :], xf)
                mrs_sb, m_row, _ = yield from ln_stats(xstat)
                t1 = sp.tile([128, DC, N], BF16, name="t_bf", tag="t_bf", bufs=3)
                nc.vector.tensor_tensor(
                    out=t1, in0=xstat[:, :, 0, :],
                    in1=mrs_sb[:, 0:N].unsqueeze(1).broadcast_to([128, DC, N]),
                    op=OP.subtract)
                nc.vector.tensor_tensor(
                    out=t1, in0=t1,
                    in1=mrs_sb[:, N:2 * N].unsqueeze(1).broadcast_to([128, DC, N]),
                    op=OP.mult)
                gsc = sp.tile([128, N], BF16, name="gsc", tag="gsc")
                for c in range(DC):
                    nc.scalar.activation(out=gsc, in_=t1[:, c, :], func=AF.Gelu,
                                         accum_out=pooledT[:, c, s:s + 1])

            GROUP = 4
            for g in range(SPC // GROUP):
                active = [sample_body(s) for s in range(g * GROUP, (g + 1) * GROUP)]
                while active:
                    nxt = []
                    for gen in active:
                        try:
                            next(gen)
                            nxt.append(gen)
                        except StopIteration:
                            pass
                    active = nxt

            # ---- classifier ----
            pbf = wp.tile([128, DC, SPC], BF16)
            nc.vector.tensor_copy(pbf, pooledT)
            logits = wp.tile([SPC, NC_], F32)
            for nh in range(2):
                l_ps = ph.tile([SPC, 500], F32, name="l_ps", tag="ph")
                for c in range(DC):
                    nc.tensor.matmul(l_ps, lhsT=pbf[:, c, :],
                                     rhs=wc[:, c, nh * 500:(nh + 1) * 500],
                                     start=(c == 0), stop=(c == DC - 1))
                nc.scalar.copy(out=logits[:, nh * 500:(nh + 1) * 500], in_=l_ps)
            nc.sync.dma_start(out=out_d[:, :], in_=logits)

    nc.finalize()
    return nc


def _prep_weights(W1, b1, W2, b2, g1, be1, g2, be2, gr, br, Wc, bc):
    assert np.all(b1 == 0) and np.all(b2 == 0) and np.all(bc == 0), "nonzero biases unsupported"
    assert np.all(be1 == 0) and np.all(be2 == 0) and np.all(br == 0), "nonzero LN biases unsupported"
    assert np.all(g2 == 1) and np.all(gr == 1), "non-identity LN scales unsupported"
    bf = ml_dtypes.bfloat16

    w1f = g1[:, :, None] * W1                                  # [NB, D, HID]
    w1f_host = np.ascontiguousarray(
        w1f.reshape(NB, DC, 128, HID).transpose(2, 0, 1, 3)).astype(bf)
    w1bn_host = (-w1f.sum(axis=1)).astype(bf)                  # [NB, HID]
    w2_host = np.ascontiguousarray(
        W2.reshape(NB, HC, 128, D).transpose(2, 0, 1, 3)).astype(bf)
    wc_host = np.ascontiguousarray(
        (Wc / float(N)).reshape(DC, 128, NC_).transpose(1, 0, 2)).astype(bf)

    eyebig = np.zeros((128, NT, N), np.float32)
    eyea = np.zeros((128, NT, N), np.float32)
    for mt in range(NT):
        for p in range(128):
            eyebig[p, mt, mt * 128 + p] = BIGNEG
            eyea[p, mt, mt * 128 + p] = 1.0
    i128 = np.eye(128, dtype=np.float32)
    return {
        "w1f": w1f_host, "w1bn": w1bn_host, "w2": w2_host, "wc": wc_host,
        "eyebig": eyebig, "eyea": eyea.astype(np.uint8),
        "i128b": i128.astype(bf), "i128f": i128,
    }


def kernel(**inputs) -> np.ndarray:
    tokens = np.asarray(inputs["tokens"], dtype=np.float32)
    k = int(np.asarray(inputs["k"]))
    assert k == 8, f"kernel specialised for k=8, got {k}"
    assert tokens.shape == (B, N, D)

    wargs = {nm: np.asarray(inputs[nm], dtype=np.float32) for nm in
             ("W1", "b1", "W2", "b2", "g1", "be1", "g2", "be2", "gr", "br", "Wc", "bc")}
    shared = _prep_weights(**wargs)

    if "nc" not in _CACHE:
        _CACHE["nc"] = build_kernel()
    nc = _CACHE["nc"]

    xT = _round_f32r(np.ascontiguousarray(tokens.transpose(0, 2, 1)))  # [B, D, N]
    in_maps = []
    for m in range(NCORES):
        im = dict(shared)
        im["xT"] = np.ascontiguousarray(xT[m * SPC:(m + 1) * SPC])
        in_maps.append(im)

    res = run_bass_kernel_spmd(nc, in_maps, list(range(NCORES)))
    out = np.concatenate([res.results[m]["out"] for m in range(NCORES)], axis=0)
    return out.astype(np.float32)


if __name__ == "__main__":
    rng = np.random.default_rng(0)
    print("smoke build only")
    build_kernel()
    print("build OK")
